# revision 9
# baseline (speedup 1.0000x reference)
"""Trainium2 Bass kernel for nn_CaptionEmbedding (ragged double-GRU with
attention gating).

Strategy: data-parallel over batch across 8 cores (strided over the
length-sorted order so every core gets a balanced length mix). Per core a
fully-unrolled 20-step recurrence in fp16 (fp32 PSUM accumulation):
  - activations live "stacked": [128, 512] = (slot + 64*feat_half, feat%512)
  - matmul stationary operands are activations, transposed on device by the
    PE array; weights stream through the PE array
  - Whh/cWhh stored fp8 e4m3 (x32 pre-scale, undone in the gate activation
    scale); Wih/cWih fp16 x32 so all gate PSUMs share one scale
  - all weights resident in SBUF except Wih, which streams with a 4-deep
    prefetch; gi (w-GRU input projections) for step t+2 are computed as PE
    gap filler spread across step t
  - step t+1's R/Z hidden projections are issued late in step t (after h1
    is transposed) to keep the PE busy through the c-GRU combine
"""
import numpy as np
import ml_dtypes

import concourse.bass as bass
import concourse.mybir as mybir
import concourse.tile as tile
from concourse.bass_utils import run_bass_kernel_spmd
import concourse.mybir as _mybir
B, VD, QD, HD, L = 512, 2048, 1024, 1024, 20
NCORES, S = 8, 64
F32, F16 = mybir.dt.float32, mybir.dt.float16
F8 = mybir.dt.float8e4
Sig = mybir.ActivationFunctionType.Sigmoid
Tanh = mybir.ActivationFunctionType.Tanh
Relu = mybir.ActivationFunctionType.Relu
Copy = mybir.ActivationFunctionType.Copy
WSC = 32.0          # GRU weight pre-scale; undone in gate activations
INV = 1.0 / WSC

_MAX_WAITS = 1
_wait_ctr = [0]


def _split_waits(nc):
    # container neuronxcc rejects >= 2 sync waits on one instruction; move
    # extras onto same-engine nops spliced just before it
    for fn in nc.m.functions:
        for bb in fn.blocks:
            out = []
            for inst in bb.instructions:
                si = inst.sync_info
                waits = list(si.on_wait) if si and si.on_wait else []
                if len(waits) > _MAX_WAITS:
                    extra, keep = waits[:-_MAX_WAITS], waits[-_MAX_WAITS:]
                    for i in range(0, len(extra), _MAX_WAITS):
                        _wait_ctr[0] += 1
                        nop = _mybir.InstNoOp(
                            name=f"waitsplit_nop_{_wait_ctr[0]}", ins=[], outs=[]
                        )
                        nop.engine = inst.engine
                        nop.sync_info = _mybir.SyncInfo(
                            on_wait=extra[i : i + _MAX_WAITS], on_update=[]
                        )
                        nc.register_instruction(nop)
                        out.append(nop)
                    si.on_wait = keep
                out.append(inst)
            if len(out) != len(bb.instructions):
                bb.instructions[:] = out


def _kt_slice(tT, kt):
    # stationary [128, 64] for feature ktile kt from a transposed
    # [128, 4, 128] tile: out[p, j, q] = stacked[q, j*128 + p]
    hi, j = kt // 4, kt % 4
    return tT[:, j, 64 * hi : 64 * hi + 64]


def _build():
    """Trace the per-core program (identical for all cores; SPMD)."""
    nc = bass.Bass("TRN2", dynamic_dma_scratch_size=64)
    di = {}
    inputs = [
        ("vT", [128, 16, S], F16),
        ("qT", [128, 8, S], F16),
        ("xT", [L, 128, 8, S], F16),
        ("wvT", [16, 128, HD], F16),
        ("wqT", [8, 128, HD], F16),
        ("wihT", [8, 2, 128, 1536], F16),
        ("whh8T", [8, 128, 3 * HD], F8),
        ("whT", [8, 128, HD], F16),
        ("wlT", [8, 128, HD], F16),
        ("wcihT", [8, 128, 3 * HD], F16),
        ("wcwh8T", [8, 128, 3 * HD], F8),
        ("wfT", [8, 128, HD], F16),
    ]
    for name, shape, dt in inputs:
        di[name] = nc.dram_tensor(name, shape, dt, kind="ExternalInput")
    outs_d = nc.dram_tensor("outs", [L, 128, 512], F32, kind="ExternalOutput")
    alph_d = nc.dram_tensor("alph", [L, 128, 512], F16, kind="ExternalOutput")

    with tile.TileContext(nc) as tc:
        _trace(nc, tc, di, outs_d, alph_d)
    _split_waits(nc)
    return nc


def _mm_gate(nc, psum, lhsT, w_ap, c0, start, stop):
    """One ktile's pair of matmuls for a 1024-wide gate at weight cols
    [c0, c0+1024): lo 512 -> psum[0:64], hi 512 -> psum[64:128]."""
    nc.tensor.matmul(psum[0:64, :], lhsT, w_ap[:, c0 : c0 + 512],
                     start=start, stop=stop)
    nc.tensor.matmul(psum[64:128, :], lhsT, w_ap[:, c0 + 512 : c0 + 1024],
                     start=start, stop=stop)


def _trace(nc, tc, di, outs_d, alph_d):
    import contextlib

    ctx = contextlib.ExitStack()
    with ctx:
        work = ctx.enter_context(tc.tile_pool(name="work", bufs=1))
        res1 = ctx.enter_context(tc.tile_pool(name="res1", bufs=1))

        # ---- resident weights, phase 1 (needed by steps' a/b/c) ----
        whh8_t = res1.tile([128, 8, 3 * HD], F8, tag="whh8")
        wh_t = res1.tile([128, 8, HD], F16, tag="wh")
        wl_t = res1.tile([128, 8, HD], F16, tag="wl")
        wf_t = res1.tile([128, 8, HD], F16, tag="wf")
        for w_sb, w_d in [(whh8_t, "whh8T"), (wh_t, "whT"), (wl_t, "wlT"),
                          (wf_t, "wfT")]:
            for kt in range(8):
                nc.sync.dma_start(out=w_sb[:, kt, :], in_=di[w_d][kt])

        # ---- small persistent tiles ----
        pvq_t = work.tile([128, 512], F32, tag="pvq")

        # pools (tags rotate within fixed slot counts)
        ctr = [0]

        def wtile(shape, dt, tag, bufs):
            def mk():
                ctr[0] += 1
                return work.tile(shape, dt, tag=tag, bufs=bufs,
                                 name=f"{tag}_{ctr[0]}")
            return mk

        t_xt = wtile([128, 8, S], F16, "xt", 3)
        t_wih = wtile([128, 1536], F16, "wih", 4)
        t_gi = wtile([128, 512], F16, "gi", 6)
        t_g = wtile([128, 512], F16, "g", 5)
        t_h1 = wtile([128, 512], F16, "h1", 2)
        t_h1T = wtile([128, 4, 128], F16, "h1T", 2)
        t_h2 = wtile([128, 512], F16, "h2", 2)
        t_h2T = wtile([128, 4, 128], F16, "h2T", 2)
        t_h2n = wtile([128, 512], F32, "h2n", 1)
        t_att = wtile([128, 512], F16, "att", 1)
        t_attT = wtile([128, 4, 128], F16, "attT", 2)
        t_jrl = wtile([128, 512], F16, "jrl", 1)
        t_jT = wtile([128, 4, 128], F16, "jT", 2)
        t_grc = wtile([128, 512], F16, "grc", 1)
        t_grcT = wtile([128, 4, 128], F16, "grcT", 2)
        t_cinT = wtile([128, 4, 128], F16, "cinT", 1)

        psum = ctx.enter_context(tc.tile_pool(name="psum", bufs=1, space="PSUM"))

        def ptile():
            ctr[0] += 1
            return psum.tile([128, 512], F32, tag="ps", name=f"ps_{ctr[0]}",
                             bufs=7)

        def pttile():
            ctr[0] += 1
            return psum.tile([128, 512], F16, tag="psT", name=f"psT_{ctr[0]}",
                             bufs=1)

        ident = work.tile([128, 128], F16, tag="ident")
        from concourse.masks import make_identity
        make_identity(nc, ident)

        def pe_transpose(dstT, src_f16):
            # dstT [128, 4, 128] <- transpose of stacked [128, 512] fp16
            pt = pttile()
            for j in range(4):
                nc.tensor.transpose(
                    pt[:, 128 * j : 128 * (j + 1)],
                    src_f16[:, 128 * j : 128 * (j + 1)],
                    ident,
                )
            nc.vector.tensor_copy(
                out=dstT.rearrange("p j q -> p (j q)"), in_=pt
            )

        # ---- prologue pool: v/q stationaries + streamed Wv/Wq chunks ----
        gi_tiles = {}  # step -> (giR, giZ, giIN) fp16 SBUF
        xt_tiles = {}

        def load_xt(t):
            xt = t_xt()
            nc.scalar.dma_start(out=xt, in_=di["xT"][t])
            xt_tiles[t] = xt

        gi_psum = {}

        def gi_part(u, kts):
            """Matmul part of gi (w-GRU input projections) for step u over
            the given ktile range. Independent of recurrent state: used as
            PE gap filler."""
            if u >= L:
                return
            if u not in gi_psum:
                gi_psum[u] = (ptile(), ptile(), ptile())
            R, Z, IN = gi_psum[u]
            tiles = []
            for kt in kts:
                wA = t_wih()
                nc.scalar.dma_start(out=wA, in_=di["wihT"][kt, 0])
                wB = t_wih()
                nc.scalar.dma_start(out=wB, in_=di["wihT"][kt, 1])
                tiles.append((kt, wA, wB))
            for kt, wA, wB in tiles:
                st, sp = kt == 0, kt == 7
                lhsT = xt_tiles[u][:, kt, :]
                nc.tensor.matmul(R[0:64], lhsT, wA[:, 0:512], start=st, stop=sp)
                nc.tensor.matmul(R[64:128], lhsT, wA[:, 512:1024], start=st, stop=sp)
                nc.tensor.matmul(Z[0:64], lhsT, wA[:, 1024:1536], start=st, stop=sp)
                nc.tensor.matmul(Z[64:128], lhsT, wB[:, 0:512], start=st, stop=sp)
                nc.tensor.matmul(IN[0:64], lhsT, wB[:, 512:1024], start=st, stop=sp)
                nc.tensor.matmul(IN[64:128], lhsT, wB[:, 1024:1536], start=st, stop=sp)

        def gi_finish(u):
            if u >= L:
                return
            R, Z, IN = gi_psum.pop(u)
            gr, gz, gn = t_gi(), t_gi(), t_gi()
            nc.scalar.activation(out=gr, in_=R, func=Copy)
            nc.scalar.activation(out=gz, in_=Z, func=Copy)
            nc.scalar.activation(out=gn, in_=IN, func=Copy)
            gi_tiles[u] = (gr, gz, gn)

        with tc.tile_pool(name="pre", bufs=1) as pre:
            v_t = pre.tile([128, 16, S], F16, tag="v")
            q_t = pre.tile([128, 8, S], F16, tag="q")
            nc.scalar.dma_start(out=v_t, in_=di["vT"][:])
            nc.scalar.dma_start(out=q_t, in_=di["qT"][:])
            pv = ptile()
            for kt in range(16):
                wc = pre.tile([128, HD], F16, tag="wvq", bufs=4)
                nc.scalar.dma_start(out=wc, in_=di["wvT"][kt])
                nc.tensor.matmul(pv[0:64], v_t[:, kt, :], wc[:, 0:512],
                                 start=(kt == 0), stop=False)
                nc.tensor.matmul(pv[64:128], v_t[:, kt, :], wc[:, 512:1024],
                                 start=(kt == 0), stop=False)
            for kt in range(8):
                wc = pre.tile([128, HD], F16, tag="wvq", bufs=4)
                nc.scalar.dma_start(out=wc, in_=di["wqT"][kt])
                nc.tensor.matmul(pv[0:64], q_t[:, kt, :], wc[:, 0:512],
                                 start=False, stop=(kt == 7))
                nc.tensor.matmul(pv[64:128], q_t[:, kt, :], wc[:, 512:1024],
                                 start=False, stop=(kt == 7))
            nc.vector.tensor_copy(out=pvq_t, in_=pv)

            # gi for steps 0,1 inside prologue -- one shared Wih pass
            load_xt(0)
            load_xt(1)
            gi_psum[0] = (ptile(), ptile(), ptile())
            gi_psum[1] = (ptile(), ptile(), ptile())
            for kt in range(8):
                wA = t_wih()
                nc.scalar.dma_start(out=wA, in_=di["wihT"][kt, 0])
                wB = t_wih()
                nc.scalar.dma_start(out=wB, in_=di["wihT"][kt, 1])
                st, sp = kt == 0, kt == 7
                for u in (0, 1):
                    R, Z, IN = gi_psum[u]
                    lhsT = xt_tiles[u][:, kt, :]
                    nc.tensor.matmul(R[0:64], lhsT, wA[:, 0:512], start=st, stop=sp)
                    nc.tensor.matmul(R[64:128], lhsT, wA[:, 512:1024], start=st, stop=sp)
                    nc.tensor.matmul(Z[0:64], lhsT, wA[:, 1024:1536], start=st, stop=sp)
                    nc.tensor.matmul(Z[64:128], lhsT, wB[:, 0:512], start=st, stop=sp)
                    nc.tensor.matmul(IN[0:64], lhsT, wB[:, 512:1024], start=st, stop=sp)
                    nc.tensor.matmul(IN[64:128], lhsT, wB[:, 1024:1536], start=st, stop=sp)
            gi_finish(0)
            gi_finish(1)

        # ---- resident weights, phase 2 (after prologue pool freed) ----
        res2 = ctx.enter_context(tc.tile_pool(name="res2", bufs=1))
        wcih_t = res2.tile([128, 8, 3 * HD], F16, tag="wcih")
        wcwh8_t = res2.tile([128, 8, 3 * HD], F8, tag="wcwh8")
        for w_sb, w_d in [(wcih_t, "wcihT"), (wcwh8_t, "wcwh8T")]:
            for kt in range(8):
                nc.sync.dma_start(out=w_sb[:, kt, :], in_=di[w_d][kt])

        # ---- initial state ----
        h1_prev = t_h1()
        nc.vector.memset(h1_prev, 0.0)
        h1T_prev = t_h1T()
        nc.vector.memset(h1T_prev, 0.0)
        h2_prev = t_h2()
        nc.vector.memset(h2_prev, 0.0)
        h2T_prev = t_h2T()
        nc.vector.memset(h2T_prev, 0.0)

        a_psum = {}

        # ---- main loop ----
        for t in range(L):
            if t + 2 < L:
                load_xt(t + 2)

            giR, giZ, giIN = gi_tiles.pop(t)
            xt = xt_tiles[t]

            # --- w-GRU hidden projections (a) ---
            if t in a_psum:
                Rw, Zw, HNw = a_psum.pop(t)
            else:
                Rw, Zw, HNw = ptile(), ptile(), ptile()
                for kt in range(8):
                    lhsT = _kt_slice(h1T_prev, kt)
                    st, sp = kt == 0, kt == 7
                    _mm_gate(nc, Rw, lhsT, whh8_t[:, kt, :], 0, st, sp)
                    _mm_gate(nc, Zw, lhsT, whh8_t[:, kt, :], HD, st, sp)
                    _mm_gate(nc, HNw, lhsT, whh8_t[:, kt, :], 2 * HD, st, sp)

            # --- c-GRU hidden projections (e) -- independent, fills PE ---
            Rc, Zc, HNc = ptile(), ptile(), ptile()
            for kt in range(8):
                lhsT = _kt_slice(h2T_prev, kt)
                st = kt == 0
                _mm_gate(nc, Rc, lhsT, wcwh8_t[:, kt, :], 0, st, False)
                _mm_gate(nc, Zc, lhsT, wcwh8_t[:, kt, :], HD, st, False)
                _mm_gate(nc, HNc, lhsT, wcwh8_t[:, kt, :], 2 * HD, st, kt == 7)

            # --- w-GRU combine: h' = n + z*(h - n) (psums carry x32) ---
            rs = t_g()
            nc.vector.tensor_add(out=rs, in0=Rw, in1=giR)
            rw = t_g()
            nc.scalar.activation(out=rw, in_=rs, func=Sig, scale=INV)
            t1 = t_g()
            nc.vector.tensor_mul(out=t1, in0=rw, in1=HNw)
            t2 = t_g()
            nc.vector.tensor_add(out=t2, in0=t1, in1=giIN)
            nw = t_g()
            nc.scalar.activation(out=nw, in_=t2, func=Tanh, scale=INV)
            zs = t_g()
            nc.vector.tensor_add(out=zs, in0=Zw, in1=giZ)
            zw = t_g()
            nc.scalar.activation(out=zw, in_=zs, func=Sig, scale=INV)
            f1 = t_g()
            nc.vector.tensor_sub(out=f1, in0=h1_prev, in1=nw)
            f2 = t_g()
            nc.vector.tensor_mul(out=f2, in0=zw, in1=f1)
            h1_new = t_h1()
            nc.vector.tensor_add(out=h1_new, in0=nw, in1=f2)
            h1T_new = t_h1T()
            pe_transpose(h1T_new, h1_new)

            # --- attention: joint = relu(pvq + h1 @ Wh.T) ---
            Bp = ptile()
            for kt in range(8):
                _mm_gate(nc, Bp, _kt_slice(h1T_new, kt), wh_t[:, kt, :], 0,
                         kt == 0, kt == 7)
            gi_part(t + 2, range(0, 2))
            ja = t_g()
            nc.vector.tensor_add(out=ja, in0=Bp, in1=pvq_t)
            jrl = t_jrl()
            nc.scalar.activation(out=jrl, in_=ja, func=Relu)
            jT = t_jT()
            pe_transpose(jT, jrl)

            # --- att = sigmoid(joint @ Wl.T) ---
            Cp = ptile()
            for kt in range(8):
                _mm_gate(nc, Cp, _kt_slice(jT, kt), wl_t[:, kt, :], 0,
                         kt == 0, kt == 7)
            gi_part(t + 2, range(2, 4))
            att = t_att()
            nc.scalar.activation(out=att, in_=Cp, func=Sig)
            nc.sync.dma_start(out=alph_d[t], in_=att)
            attT = t_attT()
            pe_transpose(attT, att)

            gi_part(t + 2, range(4, 6))

            # --- cin = att * x (feature-major) ---
            cinT = t_cinT()
            xt_r = xt.rearrange("p (hi j) s -> p j hi s", hi=2, j=4)
            nc.vector.tensor_mul(
                out=cinT.rearrange("p j (hi s) -> p j hi s", hi=2),
                in0=attT.rearrange("p j (hi s) -> p j hi s", hi=2),
                in1=xt_r,
            )

            # --- c-GRU input projections (d), fused into Rc/Zc psum ---
            INc = ptile()
            for kt in range(8):
                lhsT = _kt_slice(cinT, kt)
                sp = kt == 7
                _mm_gate(nc, Rc, lhsT, wcih_t[:, kt, :], 0, False, sp)
                _mm_gate(nc, Zc, lhsT, wcih_t[:, kt, :], HD, False, sp)
                _mm_gate(nc, INc, lhsT, wcih_t[:, kt, :], 2 * HD, kt == 0, sp)

            gi_part(t + 2, range(6, 8))
            gi_finish(t + 2)

            # --- next step's R/Z hidden projections: PE filler through the
            # c-GRU combine (h1T_new is ready; psums consumed at t+1) ---
            if t + 1 < L:
                aR, aZ = ptile(), ptile()
                for kt in range(8):
                    lhsT = _kt_slice(h1T_new, kt)
                    st, sp = kt == 0, kt == 7
                    _mm_gate(nc, aR, lhsT, whh8_t[:, kt, :], 0, st, sp)
                    _mm_gate(nc, aZ, lhsT, whh8_t[:, kt, :], HD, st, sp)

            # --- c-GRU combine: g' = n + z*(h - n) (psums carry x32) ---
            rc = t_g()
            nc.scalar.activation(out=rc, in_=Rc, func=Sig, scale=INV)
            t1c = t_g()
            nc.vector.tensor_mul(out=t1c, in0=rc, in1=HNc)
            t2c = t_g()
            nc.vector.tensor_add(out=t2c, in0=t1c, in1=INc)
            ncg = t_g()
            nc.scalar.activation(out=ncg, in_=t2c, func=Tanh, scale=INV)
            zc = t_g()
            nc.scalar.activation(out=zc, in_=Zc, func=Sig, scale=INV)
            g1 = t_g()
            nc.vector.tensor_sub(out=g1, in0=h2_prev, in1=ncg)
            g2 = t_g()
            nc.vector.tensor_mul(out=g2, in0=zc, in1=g1)
            grc = t_grc()
            nc.vector.tensor_add(out=grc, in0=ncg, in1=g2)
            grcT = t_grcT()
            pe_transpose(grcT, grc)

            # --- h2n = gru_c @ Wf.T (Wf resident) ---
            Fp = ptile()
            for kt in range(8):
                _mm_gate(nc, Fp, _kt_slice(grcT, kt), wf_t[:, kt, :], 0,
                         kt == 0, kt == 7)
            h2n = t_h2n()
            nc.vector.tensor_copy(out=h2n, in_=Fp)
            nc.sync.dma_start(out=outs_d[t], in_=h2n)
            h2_new = t_h2()
            nc.scalar.activation(out=h2_new, in_=Fp, func=Copy)

            # --- next step's N hidden projection: covers the h2-copy wait ---
            if t + 1 < L:
                aHN = ptile()
                for kt in range(8):
                    _mm_gate(nc, aHN, _kt_slice(h1T_new, kt),
                             whh8_t[:, kt, :], 2 * HD, kt == 0, kt == 7)
                a_psum[t + 1] = (aR, aZ, aHN)

            h2T_new = t_h2T()
            pe_transpose(h2T_new, h2_new)

            h1_prev, h1T_prev = h1_new, h1T_new
            h2_prev, h2T_prev = h2_new, h2T_new


_CACHED = {}


def _get_nc():
    if "nc" not in _CACHED:
        _CACHED["nc"] = _build()
    return _CACHED["nc"]


def _wn(V, g):
    return V * (g / np.linalg.norm(V.astype(np.float64)).astype(np.float32))


def _plainT(W):
    # [out, in] -> [in//128, 128, out] fp16
    inf = W.shape[1]
    return np.ascontiguousarray(W.T.reshape(inf // 128, 128, W.shape[0])).astype(
        np.float16
    )


def _plainT8(W):
    # [out, in] -> [in//128, 128, out] fp8 e4m3, scaled x32
    Wt = np.clip(np.asarray(W, np.float32).T * WSC, -240.0, 240.0)
    inf = W.shape[1]
    return np.ascontiguousarray(Wt.reshape(inf // 128, 128, W.shape[0])).astype(
        ml_dtypes.float8_e4m3
    )


def _prep_in_maps(inp):
    cap_len = inp["cap_len"].astype(np.int32)
    order = np.argsort(-cap_len, kind="stable")

    for bname in ["av_b", "aq_b", "ah_b", "al_b", "fc_b",
                  "w_bih", "w_bhh", "c_bih", "c_bhh"]:
        assert not np.any(inp[bname]), f"nonzero bias {bname} unsupported"

    Wv = _wn(inp["av_V"], inp["av_g"])
    Wq = _wn(inp["aq_V"], inp["aq_g"])
    Wh = _wn(inp["ah_V"], inp["ah_g"])
    Wl = _wn(inp["al_V"], inp["al_g"])
    Wf = _wn(inp["fc_V"], inp["fc_g"])

    shared = dict(
        wvT=_plainT(Wv), wqT=_plainT(Wq),
        wihT=np.ascontiguousarray(
            np.transpose(
                _plainT(inp["w_Wih"] * WSC).reshape(8, 128, 2, 1536),
                (0, 2, 1, 3),
            )
        ),
        whh8T=_plainT8(inp["w_Whh"]),
        whT=_plainT(Wh), wlT=_plainT(Wl),
        wcihT=_plainT(inp["c_Wih"] * WSC),
        wcwh8T=_plainT8(inp["c_Whh"]),
        wfT=_plainT(Wf),
    )

    v, q, caption = inp["v"], inp["q"], inp["caption"]
    in_maps = []
    for k in range(NCORES):
        pos = np.arange(S) * NCORES + k  # sorted positions of this core
        vk = v[pos].astype(np.float16)            # [S, VD]
        qk = q[pos].astype(np.float16)
        capk = caption[order[pos]].astype(np.float16)  # [S, L, QD]
        m = dict(shared)
        m["vT"] = np.ascontiguousarray(
            np.transpose(vk.T.reshape(16, 128, S), (1, 0, 2)))
        m["qT"] = np.ascontiguousarray(
            np.transpose(qk.T.reshape(8, 128, S), (1, 0, 2)))
        m["xT"] = np.ascontiguousarray(
            np.transpose(
                np.transpose(capk, (1, 2, 0)).reshape(L, 8, 128, S), (0, 2, 1, 3)
            )
        )
        in_maps.append(m)
    return in_maps


def kernel(**inputs):
    inp = {k: np.asarray(v) for k, v in inputs.items()}
    cap_len = inp["cap_len"].astype(np.int32)
    order = np.argsort(-cap_len, kind="stable")
    cl = cap_len[order]
    in_maps = _prep_in_maps(inp)

    nc = _get_nc()
    res = run_bass_kernel_spmd(nc, in_maps, core_ids=list(range(NCORES)))

    outs = np.zeros((B, L, HD), np.float32)
    alphas = np.zeros((B, L, HD), np.float32)
    for k in range(NCORES):
        pos = np.arange(S) * NCORES + k
        od = res.results[k]["outs"]  # [L, 128, 512] f32
        ad = res.results[k]["alph"].astype(np.float32)
        oc = np.concatenate([od[:, :S, :], od[:, S:, :]], axis=2)  # [L, S, HD]
        ac = np.concatenate([ad[:, :S, :], ad[:, S:, :]], axis=2)
        outs[pos] = np.transpose(oc, (1, 0, 2))
        alphas[pos] = np.transpose(ac, (1, 0, 2))

    mask = (np.arange(L)[None, :] < cl[:, None])[:, :, None]
    outs *= mask
    alphas *= mask
    return outs, alphas


# revision 18
# speedup vs baseline: 1.1344x; 1.1344x over previous
"""Trainium2 Bass kernel for nn_CaptionEmbedding (ragged double-GRU with
attention gating).

Strategy: data-parallel over batch across 8 cores (strided over the
length-sorted order so every core gets a balanced length mix). Per core a
fully-unrolled 20-step recurrence in fp16 (fp32 PSUM accumulation):
  - activations live "stacked": [128, 512] = (slot + 64*feat_half, feat%512)
  - matmul stationary operands are activations, transposed on device by the
    PE array; weights stream through the PE array
  - Whh/cWhh stored fp8 e4m3 (x32 pre-scale, undone in the gate activation
    scale); Wih/cWih fp16 x32 so all gate PSUMs share one scale
  - all weights resident in SBUF except Wih, which streams with a 4-deep
    prefetch; gi (w-GRU input projections) for step t+2 are computed as PE
    gap filler spread across step t
  - step t+1's R/Z hidden projections are issued late in step t (after h1
    is transposed) to keep the PE busy through the c-GRU combine
"""
import numpy as np
import ml_dtypes

import concourse.bass as bass
import concourse.mybir as mybir
import concourse.tile as tile
from concourse.bass_utils import run_bass_kernel_spmd
import concourse.mybir as _mybir
B, VD, QD, HD, L = 512, 2048, 1024, 1024, 20
NCORES, S = 8, 64
F32, F16 = mybir.dt.float32, mybir.dt.float16
F8 = mybir.dt.float8e4
Sig = mybir.ActivationFunctionType.Sigmoid
Tanh = mybir.ActivationFunctionType.Tanh
Relu = mybir.ActivationFunctionType.Relu
Copy = mybir.ActivationFunctionType.Copy
WSC = 32.0          # GRU weight pre-scale; undone in gate activations
INV = 1.0 / WSC

_MAX_WAITS = 1
_wait_ctr = [0]


def _split_waits(nc):
    # container neuronxcc rejects >= 2 sync waits on one instruction; move
    # extras onto same-engine nops spliced just before it
    for fn in nc.m.functions:
        for bb in fn.blocks:
            out = []
            for inst in bb.instructions:
                si = inst.sync_info
                waits = list(si.on_wait) if si and si.on_wait else []
                if len(waits) > _MAX_WAITS:
                    extra, keep = waits[:-_MAX_WAITS], waits[-_MAX_WAITS:]
                    for i in range(0, len(extra), _MAX_WAITS):
                        _wait_ctr[0] += 1
                        nop = _mybir.InstNoOp(
                            name=f"waitsplit_nop_{_wait_ctr[0]}", ins=[], outs=[]
                        )
                        nop.engine = inst.engine
                        nop.sync_info = _mybir.SyncInfo(
                            on_wait=extra[i : i + _MAX_WAITS], on_update=[]
                        )
                        nc.register_instruction(nop)
                        out.append(nop)
                    si.on_wait = keep
                out.append(inst)
            if len(out) != len(bb.instructions):
                bb.instructions[:] = out


def _kt_slice(tT, kt):
    # stationary [128, 64] for feature ktile kt from a transposed
    # [128, 4, 128] tile: out[p, j, q] = stacked[q, j*128 + p]
    hi, j = kt // 4, kt % 4
    return tT[:, j, 64 * hi : 64 * hi + 64]


def _build():
    """Trace the per-core program (identical for all cores; SPMD)."""
    nc = bass.Bass("TRN2", dynamic_dma_scratch_size=64)
    di = {}
    inputs = [
        ("vT", [128, 16, S], F16),
        ("qT", [128, 8, S], F16),
        ("xT", [L, 128, 8, S], F16),
        ("wvT", [16, 128, HD], F16),
        ("wqT", [8, 128, HD], F16),
        ("wih8T", [8, 128, 3 * HD], F8),
        ("whh8T", [8, 128, 3 * HD], F8),
        ("whT", [8, 128, HD], F16),
        ("wlT", [8, 128, HD], F16),
        ("wcihT", [8, 128, 3 * HD], F16),
        ("wcwh8T", [8, 128, 3 * HD], F8),
        ("wfT", [8, 128, HD], F16),
    ]
    for name, shape, dt in inputs:
        di[name] = nc.dram_tensor(name, shape, dt, kind="ExternalInput")
    outs_d = nc.dram_tensor("outs", [L, 128, 512], F32, kind="ExternalOutput")
    alph_d = nc.dram_tensor("alph", [L, 128, 512], F16, kind="ExternalOutput")

    with tile.TileContext(nc) as tc:
        _trace(nc, tc, di, outs_d, alph_d)
    _split_waits(nc)
    return nc


def _mm_gate(nc, psum, lhsT, w_ap, c0, start, stop):
    """One ktile's pair of matmuls for a 1024-wide gate at weight cols
    [c0, c0+1024): lo 512 -> psum[0:64], hi 512 -> psum[64:128]."""
    nc.tensor.matmul(psum[0:64, :], lhsT, w_ap[:, c0 : c0 + 512],
                     start=start, stop=stop)
    nc.tensor.matmul(psum[64:128, :], lhsT, w_ap[:, c0 + 512 : c0 + 1024],
                     start=start, stop=stop)


def _trace(nc, tc, di, outs_d, alph_d):
    import contextlib

    ctx = contextlib.ExitStack()
    with ctx:
        work = ctx.enter_context(tc.tile_pool(name="work", bufs=1))
        res1 = ctx.enter_context(tc.tile_pool(name="res1", bufs=1))

        # ---- resident weights, phase 1 (needed by steps' a/b/c) ----
        whh8_t = res1.tile([128, 8, 3 * HD], F8, tag="whh8")
        wih8_t = res1.tile([128, 8, 3 * HD], F8, tag="wih8")
        wh_t = res1.tile([128, 8, HD], F16, tag="wh")
        wl_t = res1.tile([128, 8, HD], F16, tag="wl")
        wf_t = res1.tile([128, 8, HD], F16, tag="wf")
        for w_sb, w_d in [(whh8_t, "whh8T"), (wih8_t, "wih8T"),
                          (wh_t, "whT"), (wl_t, "wlT"), (wf_t, "wfT")]:
            for kt in range(8):
                nc.sync.dma_start(out=w_sb[:, kt, :], in_=di[w_d][kt])

        # ---- small persistent tiles ----
        pvq_t = work.tile([128, 512], F32, tag="pvq")

        # pools (tags rotate within fixed slot counts)
        ctr = [0]

        def wtile(shape, dt, tag, bufs):
            def mk():
                ctr[0] += 1
                return work.tile(shape, dt, tag=tag, bufs=bufs,
                                 name=f"{tag}_{ctr[0]}")
            return mk

        t_xt = wtile([128, 8, S], F16, "xt", 3)
        t_gi = wtile([128, 512], F16, "gi", 6)
        t_g = wtile([128, 512], F16, "g", 5)
        t_h1 = wtile([128, 512], F16, "h1", 2)
        t_h1T = wtile([128, 4, 128], F16, "h1T", 2)
        t_h2 = wtile([128, 512], F16, "h2", 2)
        t_h2T = wtile([128, 4, 128], F16, "h2T", 2)
        t_h2n = wtile([128, 512], F32, "h2n", 1)
        t_att = wtile([128, 512], F16, "att", 1)
        t_attT = wtile([128, 4, 128], F16, "attT", 2)
        t_jrl = wtile([128, 512], F16, "jrl", 1)
        t_jT = wtile([128, 4, 128], F16, "jT", 2)
        t_grc = wtile([128, 512], F16, "grc", 1)
        t_grcT = wtile([128, 4, 128], F16, "grcT", 2)
        t_cinT = wtile([128, 4, 128], F16, "cinT", 1)

        psum = ctx.enter_context(tc.tile_pool(name="psum", bufs=1, space="PSUM"))

        def ptile():
            ctr[0] += 1
            return psum.tile([128, 512], F32, tag="ps", name=f"ps_{ctr[0]}",
                             bufs=7)

        def pttile():
            ctr[0] += 1
            return psum.tile([128, 512], F16, tag="psT", name=f"psT_{ctr[0]}",
                             bufs=1)

        ident = work.tile([128, 128], F16, tag="ident")
        from concourse.masks import make_identity
        make_identity(nc, ident)

        def pe_transpose(dstT, src_f16):
            # dstT [128, 4, 128] <- transpose of stacked [128, 512] fp16
            pt = pttile()
            for j in range(4):
                nc.tensor.transpose(
                    pt[:, 128 * j : 128 * (j + 1)],
                    src_f16[:, 128 * j : 128 * (j + 1)],
                    ident,
                )
            nc.vector.tensor_copy(
                out=dstT.rearrange("p j q -> p (j q)"), in_=pt
            )

        # ---- prologue pool: v/q stationaries + streamed Wv/Wq chunks ----
        gi_tiles = {}  # step -> (giR, giZ, giIN) fp16 SBUF
        xt_tiles = {}

        def load_xt(t):
            xt = t_xt()
            nc.scalar.dma_start(out=xt, in_=di["xT"][t])
            xt_tiles[t] = xt

        gi_psum = {}

        def gi_part(u, kts):
            """Matmul part of gi (w-GRU input projections) for step u over
            the given ktile range. Independent of recurrent state: used as
            PE gap filler."""
            if u >= L:
                return
            if u not in gi_psum:
                gi_psum[u] = (ptile(), ptile(), ptile())
            R, Z, IN = gi_psum[u]
            for kt in kts:
                st, sp = kt == 0, kt == 7
                lhsT = xt_tiles[u][:, kt, :]
                w = wih8_t[:, kt, :]
                _mm_gate(nc, R, lhsT, w, 0, st, sp)
                _mm_gate(nc, Z, lhsT, w, HD, st, sp)
                _mm_gate(nc, IN, lhsT, w, 2 * HD, st, sp)

        def gi_finish(u):
            if u >= L:
                return
            R, Z, IN = gi_psum.pop(u)
            gr, gz, gn = t_gi(), t_gi(), t_gi()
            nc.scalar.activation(out=gr, in_=R, func=Copy)
            nc.scalar.activation(out=gz, in_=Z, func=Copy)
            nc.scalar.activation(out=gn, in_=IN, func=Copy)
            gi_tiles[u] = (gr, gz, gn)

        with tc.tile_pool(name="pre", bufs=1) as pre:
            v_t = pre.tile([128, 16, S], F16, tag="v")
            q_t = pre.tile([128, 8, S], F16, tag="q")
            nc.scalar.dma_start(out=v_t, in_=di["vT"][:])
            nc.scalar.dma_start(out=q_t, in_=di["qT"][:])
            pv = ptile()
            for kt in range(16):
                wc = pre.tile([128, HD], F16, tag="wvq", bufs=4)
                nc.scalar.dma_start(out=wc, in_=di["wvT"][kt])
                nc.tensor.matmul(pv[0:64], v_t[:, kt, :], wc[:, 0:512],
                                 start=(kt == 0), stop=False)
                nc.tensor.matmul(pv[64:128], v_t[:, kt, :], wc[:, 512:1024],
                                 start=(kt == 0), stop=False)
            for kt in range(8):
                wc = pre.tile([128, HD], F16, tag="wvq", bufs=4)
                nc.scalar.dma_start(out=wc, in_=di["wqT"][kt])
                nc.tensor.matmul(pv[0:64], q_t[:, kt, :], wc[:, 0:512],
                                 start=False, stop=(kt == 7))
                nc.tensor.matmul(pv[64:128], q_t[:, kt, :], wc[:, 512:1024],
                                 start=False, stop=(kt == 7))
            nc.vector.tensor_copy(out=pvq_t, in_=pv)

            # gi for steps 0,1 inside prologue (Wih resident)
            load_xt(0)
            load_xt(1)
            gi_part(0, range(8))
            gi_finish(0)
            gi_part(1, range(8))
            gi_finish(1)

        # ---- resident weights, phase 2 (after prologue pool freed) ----
        res2 = ctx.enter_context(tc.tile_pool(name="res2", bufs=1))
        wcih_t = res2.tile([128, 8, 3 * HD], F16, tag="wcih")
        wcwh8_t = res2.tile([128, 8, 3 * HD], F8, tag="wcwh8")
        for w_sb, w_d in [(wcih_t, "wcihT"), (wcwh8_t, "wcwh8T")]:
            for kt in range(8):
                nc.sync.dma_start(out=w_sb[:, kt, :], in_=di[w_d][kt])

        # ---- initial state ----
        h1_prev = t_h1()
        nc.vector.memset(h1_prev, 0.0)
        h1T_prev = t_h1T()
        nc.vector.memset(h1T_prev, 0.0)
        h2_prev = t_h2()
        nc.vector.memset(h2_prev, 0.0)
        h2T_prev = t_h2T()
        nc.vector.memset(h2T_prev, 0.0)

        a_psum = {}

        # ---- main loop ----
        for t in range(L):
            if t + 2 < L:
                load_xt(t + 2)

            giR, giZ, giIN = gi_tiles.pop(t)
            xt = xt_tiles[t]

            # --- w-GRU hidden projections (a) ---
            if t in a_psum:
                Rw, Zw = a_psum.pop(t)
                HNw = ptile()
                for kt in range(8):
                    _mm_gate(nc, HNw, _kt_slice(h1T_prev, kt),
                             whh8_t[:, kt, :], 2 * HD, kt == 0, kt == 7)
            else:
                Rw, Zw, HNw = ptile(), ptile(), ptile()
                for kt in range(8):
                    lhsT = _kt_slice(h1T_prev, kt)
                    st, sp = kt == 0, kt == 7
                    _mm_gate(nc, Rw, lhsT, whh8_t[:, kt, :], 0, st, sp)
                    _mm_gate(nc, Zw, lhsT, whh8_t[:, kt, :], HD, st, sp)
                    _mm_gate(nc, HNw, lhsT, whh8_t[:, kt, :], 2 * HD, st, sp)

            # --- c-GRU hidden projections (e) -- independent, fills PE ---
            Rc, Zc, HNc = ptile(), ptile(), ptile()
            for kt in range(8):
                lhsT = _kt_slice(h2T_prev, kt)
                st = kt == 0
                _mm_gate(nc, Rc, lhsT, wcwh8_t[:, kt, :], 0, st, False)
                _mm_gate(nc, Zc, lhsT, wcwh8_t[:, kt, :], HD, st, False)
                _mm_gate(nc, HNc, lhsT, wcwh8_t[:, kt, :], 2 * HD, st, kt == 7)

            # --- w-GRU combine: h' = n + z*(h - n) (psums carry x32) ---
            rs = t_g()
            nc.vector.tensor_add(out=rs, in0=Rw, in1=giR)
            rw = t_g()
            nc.scalar.activation(out=rw, in_=rs, func=Sig, scale=INV)
            t1 = t_g()
            nc.vector.tensor_mul(out=t1, in0=rw, in1=HNw)
            t2 = t_g()
            nc.vector.tensor_add(out=t2, in0=t1, in1=giIN)
            nw = t_g()
            nc.scalar.activation(out=nw, in_=t2, func=Tanh, scale=INV)
            zs = t_g()
            nc.vector.tensor_add(out=zs, in0=Zw, in1=giZ)
            zw = t_g()
            nc.scalar.activation(out=zw, in_=zs, func=Sig, scale=INV)
            f1 = t_g()
            nc.vector.tensor_sub(out=f1, in0=h1_prev, in1=nw)
            f2 = t_g()
            nc.vector.tensor_mul(out=f2, in0=zw, in1=f1)
            h1_new = t_h1()
            nc.vector.tensor_add(out=h1_new, in0=nw, in1=f2)
            h1T_new = t_h1T()
            pe_transpose(h1T_new, h1_new)

            # --- attention: joint = relu(pvq + h1 @ Wh.T) ---
            Bp = ptile()
            for kt in range(8):
                _mm_gate(nc, Bp, _kt_slice(h1T_new, kt), wh_t[:, kt, :], 0,
                         kt == 0, kt == 7)
            gi_part(t + 2, range(0, 2))
            ja = t_g()
            nc.vector.tensor_add(out=ja, in0=Bp, in1=pvq_t)
            jrl = t_jrl()
            nc.scalar.activation(out=jrl, in_=ja, func=Relu)
            jT = t_jT()
            pe_transpose(jT, jrl)

            # --- att = sigmoid(joint @ Wl.T) ---
            Cp = ptile()
            for kt in range(8):
                _mm_gate(nc, Cp, _kt_slice(jT, kt), wl_t[:, kt, :], 0,
                         kt == 0, kt == 7)
            gi_part(t + 2, range(2, 4))
            att = t_att()
            nc.scalar.activation(out=att, in_=Cp, func=Sig)
            nc.sync.dma_start(out=alph_d[t], in_=att)
            attT = t_attT()
            pe_transpose(attT, att)

            gi_part(t + 2, range(4, 6))

            # --- cin = att * x (feature-major) ---
            cinT = t_cinT()
            xt_r = xt.rearrange("p (hi j) s -> p j hi s", hi=2, j=4)
            nc.vector.tensor_mul(
                out=cinT.rearrange("p j (hi s) -> p j hi s", hi=2),
                in0=attT.rearrange("p j (hi s) -> p j hi s", hi=2),
                in1=xt_r,
            )

            # --- c-GRU input projections (d), fused into Rc/Zc psum ---
            INc = ptile()
            for kt in range(8):
                lhsT = _kt_slice(cinT, kt)
                sp = kt == 7
                _mm_gate(nc, Rc, lhsT, wcih_t[:, kt, :], 0, False, sp)
                _mm_gate(nc, Zc, lhsT, wcih_t[:, kt, :], HD, False, sp)
                _mm_gate(nc, INc, lhsT, wcih_t[:, kt, :], 2 * HD, kt == 0, sp)

            gi_part(t + 2, range(6, 8))
            gi_finish(t + 2)

            # --- next step's R/Z hidden projections: PE filler through the
            # c-GRU combine (h1T_new is ready; psums consumed at t+1) ---
            if t + 1 < L:
                aR, aZ = ptile(), ptile()
                for kt in range(8):
                    lhsT = _kt_slice(h1T_new, kt)
                    st, sp = kt == 0, kt == 7
                    _mm_gate(nc, aR, lhsT, whh8_t[:, kt, :], 0, st, sp)
                    _mm_gate(nc, aZ, lhsT, whh8_t[:, kt, :], HD, st, sp)
                a_psum[t + 1] = (aR, aZ)

            # --- c-GRU combine: g' = n + z*(h - n) (psums carry x32) ---
            rc = t_g()
            nc.scalar.activation(out=rc, in_=Rc, func=Sig, scale=INV)
            t1c = t_g()
            nc.vector.tensor_mul(out=t1c, in0=rc, in1=HNc)
            t2c = t_g()
            nc.vector.tensor_add(out=t2c, in0=t1c, in1=INc)
            ncg = t_g()
            nc.scalar.activation(out=ncg, in_=t2c, func=Tanh, scale=INV)
            zc = t_g()
            nc.scalar.activation(out=zc, in_=Zc, func=Sig, scale=INV)
            g1 = t_g()
            nc.vector.tensor_sub(out=g1, in0=h2_prev, in1=ncg)
            g2 = t_g()
            nc.vector.tensor_mul(out=g2, in0=zc, in1=g1)
            grc = t_grc()
            nc.vector.tensor_add(out=grc, in0=ncg, in1=g2)
            grcT = t_grcT()
            pe_transpose(grcT, grc)

            # --- h2n = gru_c @ Wf.T (Wf resident) ---
            Fp = ptile()
            for kt in range(8):
                _mm_gate(nc, Fp, _kt_slice(grcT, kt), wf_t[:, kt, :], 0,
                         kt == 0, kt == 7)
            h2n = t_h2n()
            nc.vector.tensor_copy(out=h2n, in_=Fp)
            nc.sync.dma_start(out=outs_d[t], in_=h2n)
            h2_new = t_h2()
            nc.scalar.activation(out=h2_new, in_=Fp, func=Copy)
            h2T_new = t_h2T()
            pe_transpose(h2T_new, h2_new)

            h1_prev, h1T_prev = h1_new, h1T_new
            h2_prev, h2T_prev = h2_new, h2T_new


_CACHED = {}


def _get_nc():
    if "nc" not in _CACHED:
        _CACHED["nc"] = _build()
    return _CACHED["nc"]


def _wn(V, g):
    return V * (g / np.linalg.norm(V.astype(np.float64)).astype(np.float32))


def _plainT(W):
    # [out, in] -> [in//128, 128, out] fp16
    inf = W.shape[1]
    return np.ascontiguousarray(W.T.reshape(inf // 128, 128, W.shape[0])).astype(
        np.float16
    )


def _plainT8(W):
    # [out, in] -> [in//128, 128, out] fp8 e4m3, scaled x32
    Wt = np.clip(np.asarray(W, np.float32).T * WSC, -240.0, 240.0)
    inf = W.shape[1]
    return np.ascontiguousarray(Wt.reshape(inf // 128, 128, W.shape[0])).astype(
        ml_dtypes.float8_e4m3
    )


def _prep_in_maps(inp):
    cap_len = inp["cap_len"].astype(np.int32)
    order = np.argsort(-cap_len, kind="stable")

    for bname in ["av_b", "aq_b", "ah_b", "al_b", "fc_b",
                  "w_bih", "w_bhh", "c_bih", "c_bhh"]:
        assert not np.any(inp[bname]), f"nonzero bias {bname} unsupported"

    Wv = _wn(inp["av_V"], inp["av_g"])
    Wq = _wn(inp["aq_V"], inp["aq_g"])
    Wh = _wn(inp["ah_V"], inp["ah_g"])
    Wl = _wn(inp["al_V"], inp["al_g"])
    Wf = _wn(inp["fc_V"], inp["fc_g"])

    shared = dict(
        wvT=_plainT(Wv), wqT=_plainT(Wq),
        wih8T=_plainT8(inp["w_Wih"]),
        whh8T=_plainT8(inp["w_Whh"]),
        whT=_plainT(Wh), wlT=_plainT(Wl),
        wcihT=_plainT(inp["c_Wih"] * WSC),
        wcwh8T=_plainT8(inp["c_Whh"]),
        wfT=_plainT(Wf),
    )

    v, q, caption = inp["v"], inp["q"], inp["caption"]
    in_maps = []
    for k in range(NCORES):
        pos = np.arange(S) * NCORES + k  # sorted positions of this core
        vk = v[pos].astype(np.float16)            # [S, VD]
        qk = q[pos].astype(np.float16)
        capk = caption[order[pos]].astype(np.float16)  # [S, L, QD]
        m = dict(shared)
        m["vT"] = np.ascontiguousarray(
            np.transpose(vk.T.reshape(16, 128, S), (1, 0, 2)))
        m["qT"] = np.ascontiguousarray(
            np.transpose(qk.T.reshape(8, 128, S), (1, 0, 2)))
        m["xT"] = np.ascontiguousarray(
            np.transpose(
                np.transpose(capk, (1, 2, 0)).reshape(L, 8, 128, S), (0, 2, 1, 3)
            )
        )
        in_maps.append(m)
    return in_maps


def kernel(**inputs):
    inp = {k: np.asarray(v) for k, v in inputs.items()}
    cap_len = inp["cap_len"].astype(np.int32)
    order = np.argsort(-cap_len, kind="stable")
    cl = cap_len[order]
    in_maps = _prep_in_maps(inp)

    nc = _get_nc()
    res = run_bass_kernel_spmd(nc, in_maps, core_ids=list(range(NCORES)))

    outs = np.zeros((B, L, HD), np.float32)
    alphas = np.zeros((B, L, HD), np.float32)
    for k in range(NCORES):
        pos = np.arange(S) * NCORES + k
        od = res.results[k]["outs"]  # [L, 128, 512] f32
        ad = res.results[k]["alph"].astype(np.float32)
        oc = np.concatenate([od[:, :S, :], od[:, S:, :]], axis=2)  # [L, S, HD]
        ac = np.concatenate([ad[:, :S, :], ad[:, S:, :]], axis=2)
        outs[pos] = np.transpose(oc, (1, 0, 2))
        alphas[pos] = np.transpose(ac, (1, 0, 2))

    mask = (np.arange(L)[None, :] < cl[:, None])[:, :, None]
    outs *= mask
    alphas *= mask
    return outs, alphas


# revision 21
# speedup vs baseline: 1.2501x; 1.1020x over previous
"""Trainium2 Bass kernel for nn_CaptionEmbedding (ragged double-GRU with
attention gating).

Strategy: data-parallel over batch across 8 cores (strided over the
length-sorted order so every core gets a balanced length mix). Per core a
fully-unrolled 20-step recurrence in fp16 (fp32 PSUM accumulation):
  - activations live "stacked": [128, 512] = (slot + 64*feat_half, feat%512)
  - matmul stationary operands are activations, transposed on device by the
    PE array; weights stream through the PE array
  - Whh/cWhh stored fp8 e4m3 (x32 pre-scale, undone in the gate activation
    scale); Wih/cWih fp16 x32 so all gate PSUMs share one scale
  - all weights resident in SBUF except Wih, which streams with a 4-deep
    prefetch; gi (w-GRU input projections) for step t+2 are computed as PE
    gap filler spread across step t
  - step t+1's R/Z hidden projections are issued late in step t (after h1
    is transposed) to keep the PE busy through the c-GRU combine
"""
import numpy as np
import ml_dtypes

import concourse.bass as bass
import concourse.mybir as mybir
import concourse.tile as tile
from concourse.bass_utils import run_bass_kernel_spmd
import concourse.mybir as _mybir
B, VD, QD, HD, L = 512, 2048, 1024, 1024, 20
NCORES, S = 8, 64
F32, F16 = mybir.dt.float32, mybir.dt.float16
F8 = mybir.dt.float8e4
Sig = mybir.ActivationFunctionType.Sigmoid
Tanh = mybir.ActivationFunctionType.Tanh
Relu = mybir.ActivationFunctionType.Relu
Copy = mybir.ActivationFunctionType.Copy
WSC = 32.0          # GRU weight pre-scale; undone in gate activations
INV = 1.0 / WSC

_MAX_WAITS = 1
_wait_ctr = [0]


def _split_waits(nc):
    # container neuronxcc rejects >= 2 sync waits on one instruction; move
    # extras onto same-engine nops spliced just before it
    for fn in nc.m.functions:
        for bb in fn.blocks:
            out = []
            for inst in bb.instructions:
                si = inst.sync_info
                waits = list(si.on_wait) if si and si.on_wait else []
                if len(waits) > _MAX_WAITS:
                    extra, keep = waits[:-_MAX_WAITS], waits[-_MAX_WAITS:]
                    for i in range(0, len(extra), _MAX_WAITS):
                        _wait_ctr[0] += 1
                        nop = _mybir.InstNoOp(
                            name=f"waitsplit_nop_{_wait_ctr[0]}", ins=[], outs=[]
                        )
                        nop.engine = inst.engine
                        nop.sync_info = _mybir.SyncInfo(
                            on_wait=extra[i : i + _MAX_WAITS], on_update=[]
                        )
                        nc.register_instruction(nop)
                        out.append(nop)
                    si.on_wait = keep
                out.append(inst)
            if len(out) != len(bb.instructions):
                bb.instructions[:] = out


def _kt_slice(tT, kt):
    # stationary [128, 64] for feature ktile kt from a transposed
    # [128, 4, 128] tile: out[p, j, q] = stacked[q, j*128 + p]
    hi, j = kt // 4, kt % 4
    return tT[:, j, 64 * hi : 64 * hi + 64]


def _build():
    """Trace the per-core program (identical for all cores; SPMD)."""
    nc = bass.Bass("TRN2", dynamic_dma_scratch_size=64)
    di = {}
    inputs = [
        ("vT", [128, 16, S], F16),
        ("qT", [128, 8, S], F16),
        ("xT", [L, 128, 8, S], F16),
        ("wvT", [16, 128, HD], F16),
        ("wqT", [8, 128, HD], F16),
        ("wih8T", [8, 128, 3 * HD], F8),
        ("whh8T", [8, 128, 3 * HD], F8),
        ("whT", [8, 128, HD], F16),
        ("wlT", [8, 128, HD], F16),
        ("wcihT", [8, 128, 3 * HD], F16),
        ("wcwh8T", [8, 128, 3 * HD], F8),
        ("wfT", [8, 128, HD], F16),
    ]
    for name, shape, dt in inputs:
        di[name] = nc.dram_tensor(name, shape, dt, kind="ExternalInput")
    outs_d = nc.dram_tensor("outs", [L, 128, 512], F32, kind="ExternalOutput")
    alph_d = nc.dram_tensor("alph", [L, 128, 512], F16, kind="ExternalOutput")

    with tile.TileContext(nc) as tc:
        _trace(nc, tc, di, outs_d, alph_d)
    _split_waits(nc)
    return nc


def _mm_gate(nc, psum, lhsT, w_ap, c0, start, stop):
    """One ktile's pair of matmuls for a 1024-wide gate at weight cols
    [c0, c0+1024): lo 512 -> psum[0:64], hi 512 -> psum[64:128]."""
    nc.tensor.matmul(psum[0:64, :], lhsT, w_ap[:, c0 : c0 + 512],
                     start=start, stop=stop)
    nc.tensor.matmul(psum[64:128, :], lhsT, w_ap[:, c0 + 512 : c0 + 1024],
                     start=start, stop=stop)


def _trace(nc, tc, di, outs_d, alph_d):
    import contextlib

    ctx = contextlib.ExitStack()
    with ctx:
        work = ctx.enter_context(tc.tile_pool(name="work", bufs=1))
        res1 = ctx.enter_context(tc.tile_pool(name="res1", bufs=1))

        # ---- resident weights, phase 1 (needed by steps' a/b/c) ----
        whh8_t = res1.tile([128, 8, 3 * HD], F8, tag="whh8")
        wih8_t = res1.tile([128, 8, 3 * HD], F8, tag="wih8")
        wcwh8_t = res1.tile([128, 8, 3 * HD], F8, tag="wcwh8")
        wh_t = res1.tile([128, 8, HD], F16, tag="wh")
        wl_t = res1.tile([128, 8, HD], F16, tag="wl")
        wf_t = res1.tile([128, 8, HD], F16, tag="wf")
        for w_sb, w_d in [(whh8_t, "whh8T"), (wih8_t, "wih8T"),
                          (wcwh8_t, "wcwh8T"),
                          (wh_t, "whT"), (wl_t, "wlT"), (wf_t, "wfT")]:
            for kt in range(8):
                nc.sync.dma_start(out=w_sb[:, kt, :], in_=di[w_d][kt])

        # ---- small persistent tiles ----
        pvq_t = work.tile([128, 512], F32, tag="pvq")

        # pools (tags rotate within fixed slot counts)
        ctr = [0]

        def wtile(shape, dt, tag, bufs):
            def mk():
                ctr[0] += 1
                return work.tile(shape, dt, tag=tag, bufs=bufs,
                                 name=f"{tag}_{ctr[0]}")
            return mk

        t_xt = wtile([128, 8, S], F16, "xt", 3)
        t_gi = wtile([128, 512], F16, "gi", 6)
        t_g = wtile([128, 512], F16, "g", 5)
        t_h1 = wtile([128, 512], F16, "h1", 2)
        t_h1T = wtile([128, 4, 128], F16, "h1T", 2)
        t_h2 = wtile([128, 512], F16, "h2", 2)
        t_h2T = wtile([128, 4, 128], F16, "h2T", 2)
        t_h2n = wtile([128, 512], F32, "h2n", 1)
        t_att = wtile([128, 512], F16, "att", 1)
        t_attT = wtile([128, 4, 128], F16, "attT", 2)
        t_jrl = wtile([128, 512], F16, "jrl", 1)
        t_jT = wtile([128, 4, 128], F16, "jT", 2)
        t_grc = wtile([128, 512], F16, "grc", 1)
        t_grcT = wtile([128, 4, 128], F16, "grcT", 2)
        t_cinT = wtile([128, 4, 128], F16, "cinT", 1)

        psum = ctx.enter_context(tc.tile_pool(name="psum", bufs=1, space="PSUM"))

        def ptile():
            ctr[0] += 1
            return psum.tile([128, 512], F32, tag="ps", name=f"ps_{ctr[0]}",
                             bufs=7)

        def pttile():
            ctr[0] += 1
            return psum.tile([128, 512], F16, tag="psT", name=f"psT_{ctr[0]}",
                             bufs=1)

        ident = work.tile([128, 128], F16, tag="ident")
        from concourse.masks import make_identity
        make_identity(nc, ident)

        def pe_transpose(dstT, src_f16):
            # dstT [128, 4, 128] <- transpose of stacked [128, 512] fp16
            pt = pttile()
            for j in range(4):
                nc.tensor.transpose(
                    pt[:, 128 * j : 128 * (j + 1)],
                    src_f16[:, 128 * j : 128 * (j + 1)],
                    ident,
                )
            nc.vector.tensor_copy(
                out=dstT.rearrange("p j q -> p (j q)"), in_=pt
            )

        # ---- prologue pool: v/q stationaries + streamed Wv/Wq chunks ----
        gi_tiles = {}  # step -> (giR, giZ, giIN) fp16 SBUF
        xt_tiles = {}

        def load_xt(t):
            xt = t_xt()
            nc.scalar.dma_start(out=xt, in_=di["xT"][t])
            xt_tiles[t] = xt

        gi_psum = {}

        def gi_part(u, kts):
            """Matmul part of gi (w-GRU input projections) for step u over
            the given ktile range. Independent of recurrent state: used as
            PE gap filler."""
            if u >= L:
                return
            if u not in gi_psum:
                gi_psum[u] = (ptile(), ptile(), ptile())
            R, Z, IN = gi_psum[u]
            for kt in kts:
                st, sp = kt == 0, kt == 7
                lhsT = xt_tiles[u][:, kt, :]
                w = wih8_t[:, kt, :]
                _mm_gate(nc, R, lhsT, w, 0, st, sp)
                _mm_gate(nc, Z, lhsT, w, HD, st, sp)
                _mm_gate(nc, IN, lhsT, w, 2 * HD, st, sp)

        def gi_finish(u):
            if u >= L:
                return
            R, Z, IN = gi_psum.pop(u)
            gr, gz, gn = t_gi(), t_gi(), t_gi()
            nc.scalar.activation(out=gr, in_=R, func=Copy)
            nc.scalar.activation(out=gz, in_=Z, func=Copy)
            nc.scalar.activation(out=gn, in_=IN, func=Copy)
            gi_tiles[u] = (gr, gz, gn)

        with tc.tile_pool(name="pre", bufs=1) as pre:
            v_t = pre.tile([128, 16, S], F16, tag="v")
            q_t = pre.tile([128, 8, S], F16, tag="q")
            nc.scalar.dma_start(out=v_t, in_=di["vT"][:])
            nc.scalar.dma_start(out=q_t, in_=di["qT"][:])
            pv = ptile()
            for kt in range(16):
                wc = pre.tile([128, HD], F16, tag="wvq", bufs=6)
                nc.scalar.dma_start(out=wc, in_=di["wvT"][kt])
                nc.tensor.matmul(pv[0:64], v_t[:, kt, :], wc[:, 0:512],
                                 start=(kt == 0), stop=False)
                nc.tensor.matmul(pv[64:128], v_t[:, kt, :], wc[:, 512:1024],
                                 start=(kt == 0), stop=False)
            for kt in range(8):
                wc = pre.tile([128, HD], F16, tag="wvq", bufs=6)
                nc.scalar.dma_start(out=wc, in_=di["wqT"][kt])
                nc.tensor.matmul(pv[0:64], q_t[:, kt, :], wc[:, 0:512],
                                 start=False, stop=(kt == 7))
                nc.tensor.matmul(pv[64:128], q_t[:, kt, :], wc[:, 512:1024],
                                 start=False, stop=(kt == 7))
            nc.vector.tensor_copy(out=pvq_t, in_=pv)

            # gi for steps 0,1 inside prologue (Wih resident)
            load_xt(0)
            load_xt(1)
            gi_part(0, range(8))
            gi_finish(0)
            gi_part(1, range(8))
            gi_finish(1)

        # ---- resident weights, phase 2 (after prologue pool freed) ----
        res2 = ctx.enter_context(tc.tile_pool(name="res2", bufs=1))
        wcih_t = res2.tile([128, 8, 3 * HD], F16, tag="wcih")
        for kt in range(8):
            nc.sync.dma_start(out=wcih_t[:, kt, :], in_=di["wcihT"][kt])

        # ---- initial state ----
        h1_prev = t_h1()
        nc.vector.memset(h1_prev, 0.0)
        h1T_prev = t_h1T()
        nc.vector.memset(h1T_prev, 0.0)
        h2_prev = t_h2()
        nc.vector.memset(h2_prev, 0.0)
        h2T_prev = t_h2T()
        nc.vector.memset(h2T_prev, 0.0)

        a_psum = {}

        # ---- main loop ----
        for t in range(L):
            if t + 2 < L:
                load_xt(t + 2)

            giR, giZ, giIN = gi_tiles.pop(t)
            xt = xt_tiles[t]

            # --- w-GRU hidden projections (a) ---
            if t in a_psum:
                Rw, Zw = a_psum.pop(t)
                HNw = ptile()
                for kt in range(8):
                    _mm_gate(nc, HNw, _kt_slice(h1T_prev, kt),
                             whh8_t[:, kt, :], 2 * HD, kt == 0, kt == 7)
            else:
                Rw, Zw, HNw = ptile(), ptile(), ptile()
                for kt in range(8):
                    lhsT = _kt_slice(h1T_prev, kt)
                    st, sp = kt == 0, kt == 7
                    _mm_gate(nc, Rw, lhsT, whh8_t[:, kt, :], 0, st, sp)
                    _mm_gate(nc, Zw, lhsT, whh8_t[:, kt, :], HD, st, sp)
                    _mm_gate(nc, HNw, lhsT, whh8_t[:, kt, :], 2 * HD, st, sp)

            # --- c-GRU hidden projections (e) -- independent, fills PE ---
            Rc, Zc, HNc = ptile(), ptile(), ptile()
            for kt in range(8):
                lhsT = _kt_slice(h2T_prev, kt)
                st = kt == 0
                _mm_gate(nc, Rc, lhsT, wcwh8_t[:, kt, :], 0, st, False)
                _mm_gate(nc, Zc, lhsT, wcwh8_t[:, kt, :], HD, st, False)
                _mm_gate(nc, HNc, lhsT, wcwh8_t[:, kt, :], 2 * HD, st, kt == 7)

            # --- w-GRU combine: h' = n + z*(h - n) (psums carry x32) ---
            rs = t_g()
            nc.vector.tensor_add(out=rs, in0=Rw, in1=giR)
            rw = t_g()
            nc.scalar.activation(out=rw, in_=rs, func=Sig, scale=INV)
            t1 = t_g()
            nc.vector.tensor_mul(out=t1, in0=rw, in1=HNw)
            t2 = t_g()
            nc.vector.tensor_add(out=t2, in0=t1, in1=giIN)
            nw = t_g()
            nc.scalar.activation(out=nw, in_=t2, func=Tanh, scale=INV)
            zs = t_g()
            nc.vector.tensor_add(out=zs, in0=Zw, in1=giZ)
            zw = t_g()
            nc.scalar.activation(out=zw, in_=zs, func=Sig, scale=INV)
            f1 = t_g()
            nc.vector.tensor_sub(out=f1, in0=h1_prev, in1=nw)
            f2 = t_g()
            nc.vector.tensor_mul(out=f2, in0=zw, in1=f1)
            h1_new = t_h1()
            nc.vector.tensor_add(out=h1_new, in0=nw, in1=f2)
            h1T_new = t_h1T()
            pe_transpose(h1T_new, h1_new)

            # --- attention: joint = relu(pvq + h1 @ Wh.T) ---
            Bp = ptile()
            for kt in range(8):
                _mm_gate(nc, Bp, _kt_slice(h1T_new, kt), wh_t[:, kt, :], 0,
                         kt == 0, kt == 7)
            gi_part(t + 2, range(0, 2))
            ja = t_g()
            nc.vector.tensor_add(out=ja, in0=Bp, in1=pvq_t)
            jrl = t_jrl()
            nc.scalar.activation(out=jrl, in_=ja, func=Relu)
            jT = t_jT()
            pe_transpose(jT, jrl)

            # --- att = sigmoid(joint @ Wl.T) ---
            Cp = ptile()
            for kt in range(8):
                _mm_gate(nc, Cp, _kt_slice(jT, kt), wl_t[:, kt, :], 0,
                         kt == 0, kt == 7)
            gi_part(t + 2, range(2, 4))
            att = t_att()
            nc.scalar.activation(out=att, in_=Cp, func=Sig)
            nc.sync.dma_start(out=alph_d[t], in_=att)
            attT = t_attT()
            pe_transpose(attT, att)

            gi_part(t + 2, range(4, 6))

            # --- cin = att * x (feature-major) ---
            cinT = t_cinT()
            xt_r = xt.rearrange("p (hi j) s -> p j hi s", hi=2, j=4)
            nc.vector.tensor_mul(
                out=cinT.rearrange("p j (hi s) -> p j hi s", hi=2),
                in0=attT.rearrange("p j (hi s) -> p j hi s", hi=2),
                in1=xt_r,
            )

            # --- c-GRU input projections (d), fused into Rc/Zc psum ---
            INc = ptile()
            for kt in range(8):
                lhsT = _kt_slice(cinT, kt)
                sp = kt == 7
                _mm_gate(nc, Rc, lhsT, wcih_t[:, kt, :], 0, False, sp)
                _mm_gate(nc, Zc, lhsT, wcih_t[:, kt, :], HD, False, sp)
                _mm_gate(nc, INc, lhsT, wcih_t[:, kt, :], 2 * HD, kt == 0, sp)

            gi_part(t + 2, range(6, 8))
            gi_finish(t + 2)

            # --- next step's R/Z hidden projections: PE filler through the
            # c-GRU combine (h1T_new is ready; psums consumed at t+1) ---
            if t + 1 < L:
                aR, aZ = ptile(), ptile()
                for kt in range(8):
                    lhsT = _kt_slice(h1T_new, kt)
                    st, sp = kt == 0, kt == 7
                    _mm_gate(nc, aR, lhsT, whh8_t[:, kt, :], 0, st, sp)
                    _mm_gate(nc, aZ, lhsT, whh8_t[:, kt, :], HD, st, sp)
                a_psum[t + 1] = (aR, aZ)

            # --- c-GRU combine: g' = n + z*(h - n) (psums carry x32) ---
            rc = t_g()
            nc.scalar.activation(out=rc, in_=Rc, func=Sig, scale=INV)
            t1c = t_g()
            nc.vector.tensor_mul(out=t1c, in0=rc, in1=HNc)
            t2c = t_g()
            nc.vector.tensor_add(out=t2c, in0=t1c, in1=INc)
            ncg = t_g()
            nc.scalar.activation(out=ncg, in_=t2c, func=Tanh, scale=INV)
            zc = t_g()
            nc.scalar.activation(out=zc, in_=Zc, func=Sig, scale=INV)
            g1 = t_g()
            nc.vector.tensor_sub(out=g1, in0=h2_prev, in1=ncg)
            g2 = t_g()
            nc.vector.tensor_mul(out=g2, in0=zc, in1=g1)
            grc = t_grc()
            nc.vector.tensor_add(out=grc, in0=ncg, in1=g2)
            grcT = t_grcT()
            pe_transpose(grcT, grc)

            # --- h2n = gru_c @ Wf.T (Wf resident) ---
            Fp = ptile()
            for kt in range(8):
                _mm_gate(nc, Fp, _kt_slice(grcT, kt), wf_t[:, kt, :], 0,
                         kt == 0, kt == 7)
            h2n = t_h2n()
            nc.vector.tensor_copy(out=h2n, in_=Fp)
            nc.sync.dma_start(out=outs_d[t], in_=h2n)
            h2_new = t_h2()
            nc.scalar.activation(out=h2_new, in_=Fp, func=Copy)
            h2T_new = t_h2T()
            pe_transpose(h2T_new, h2_new)

            h1_prev, h1T_prev = h1_new, h1T_new
            h2_prev, h2T_prev = h2_new, h2T_new


_CACHED = {}


def _get_nc():
    if "nc" not in _CACHED:
        _CACHED["nc"] = _build()
    return _CACHED["nc"]


def _wn(V, g):
    return V * (g / np.linalg.norm(V.astype(np.float64)).astype(np.float32))


def _plainT(W):
    # [out, in] -> [in//128, 128, out] fp16
    inf = W.shape[1]
    return np.ascontiguousarray(W.T.reshape(inf // 128, 128, W.shape[0])).astype(
        np.float16
    )


def _plainT8(W):
    # [out, in] -> [in//128, 128, out] fp8 e4m3, scaled x32
    Wt = np.clip(np.asarray(W, np.float32).T * WSC, -240.0, 240.0)
    inf = W.shape[1]
    return np.ascontiguousarray(Wt.reshape(inf // 128, 128, W.shape[0])).astype(
        ml_dtypes.float8_e4m3
    )


def _prep_in_maps(inp):
    cap_len = inp["cap_len"].astype(np.int32)
    order = np.argsort(-cap_len, kind="stable")

    for bname in ["av_b", "aq_b", "ah_b", "al_b", "fc_b",
                  "w_bih", "w_bhh", "c_bih", "c_bhh"]:
        assert not np.any(inp[bname]), f"nonzero bias {bname} unsupported"

    Wv = _wn(inp["av_V"], inp["av_g"])
    Wq = _wn(inp["aq_V"], inp["aq_g"])
    Wh = _wn(inp["ah_V"], inp["ah_g"])
    Wl = _wn(inp["al_V"], inp["al_g"])
    Wf = _wn(inp["fc_V"], inp["fc_g"])

    shared = dict(
        wvT=_plainT(Wv), wqT=_plainT(Wq),
        wih8T=_plainT8(inp["w_Wih"]),
        whh8T=_plainT8(inp["w_Whh"]),
        whT=_plainT(Wh), wlT=_plainT(Wl),
        wcihT=_plainT(inp["c_Wih"] * WSC),
        wcwh8T=_plainT8(inp["c_Whh"]),
        wfT=_plainT(Wf),
    )

    v, q, caption = inp["v"], inp["q"], inp["caption"]
    in_maps = []
    for k in range(NCORES):
        pos = np.arange(S) * NCORES + k  # sorted positions of this core
        vk = v[pos].astype(np.float16)            # [S, VD]
        qk = q[pos].astype(np.float16)
        capk = caption[order[pos]].astype(np.float16)  # [S, L, QD]
        m = dict(shared)
        m["vT"] = np.ascontiguousarray(
            np.transpose(vk.T.reshape(16, 128, S), (1, 0, 2)))
        m["qT"] = np.ascontiguousarray(
            np.transpose(qk.T.reshape(8, 128, S), (1, 0, 2)))
        m["xT"] = np.ascontiguousarray(
            np.transpose(
                np.transpose(capk, (1, 2, 0)).reshape(L, 8, 128, S), (0, 2, 1, 3)
            )
        )
        in_maps.append(m)
    return in_maps


def kernel(**inputs):
    inp = {k: np.asarray(v) for k, v in inputs.items()}
    cap_len = inp["cap_len"].astype(np.int32)
    order = np.argsort(-cap_len, kind="stable")
    cl = cap_len[order]
    in_maps = _prep_in_maps(inp)

    nc = _get_nc()
    res = run_bass_kernel_spmd(nc, in_maps, core_ids=list(range(NCORES)))

    outs = np.zeros((B, L, HD), np.float32)
    alphas = np.zeros((B, L, HD), np.float32)
    for k in range(NCORES):
        pos = np.arange(S) * NCORES + k
        od = res.results[k]["outs"]  # [L, 128, 512] f32
        ad = res.results[k]["alph"].astype(np.float32)
        oc = np.concatenate([od[:, :S, :], od[:, S:, :]], axis=2)  # [L, S, HD]
        ac = np.concatenate([ad[:, :S, :], ad[:, S:, :]], axis=2)
        outs[pos] = np.transpose(oc, (1, 0, 2))
        alphas[pos] = np.transpose(ac, (1, 0, 2))

    mask = (np.arange(L)[None, :] < cl[:, None])[:, :, None]
    outs *= mask
    alphas *= mask
    return outs, alphas
